# revision 18
# baseline (speedup 1.0000x reference)
"""Trainium2 kernel for nn_PlaneElement (kinematic-wave plane element step).

The reference returns only 3 scalars: [outflow_q, infil_rate, infil_depth].
The only part that touches the full 4M-element `area` tensor is the global
mean (Green-Ampt surface head) — a 16 MB f32 reduction.  Everything else is
O(1) scalar math plus a 3-point MUSCL stencil at the outlet node.

Strategy:
  * Shard `area` 1-D across the 8 NeuronCores (500k elements each).
  * Each core streams its shard HBM->SBUF and reduces it to per-partition
    partial sums ([128 x n_chunks] f32), which are DMA'd back out.
  * Host gathers the 8 x 128 x n_chunks partials (plus a 32-element tail per
    shard that doesn't fit the 128-partition tiling) and finishes the scalar
    infiltration + outlet-stencil epilogue in float64.
"""

import numpy as np

N = 4_000_000
NCORES = 8
SHARD = N // NCORES            # 500_000 elements per core
P = 128                        # SBUF partitions
F = SHARD // P                 # 3906 columns per core on device
DEV_ELEMS = P * F              # 499_968
TAIL = SHARD - DEV_ELEMS       # 32 leftover elements per shard (host-summed)
EPS = 1e-9

# free-dim chunk widths: decreasing sizes so the per-chunk DVE reduces keep
# up with the DMA stream and the last reduce trails it by as little as
# possible; earlier chunks stay big for large DMA descriptors (width*4 B)
CHUNK_WIDTHS = (1280, 1024, 768, 512, 194, 128)
assert sum(CHUNK_WIDTHS) == F
# issue loads alternately on both HWDGE rings (sync + scalar engines)
DUAL_RING = False
# wait for the output DMA's completion semaphore before ending the block;
# not needed: the 12-byte store lands ~2 us after issue, well inside the
# ~7.5 us fixed NEFF quiesce trailer that follows the block
WAIT_OUT = False

_CACHE = {}


def _chunk_bounds():
    bounds = [0]
    for w in CHUNK_WIDTHS:
        bounds.append(bounds[-1] + w)
    return list(zip(bounds[:-1], bounds[1:]))


def _build_program():
    from contextlib import ExitStack

    from concourse import bacc, mybir

    chunks = _chunk_bounds()
    nch = len(chunks)
    # Raw Bacc with manual semaphores: TileContext's entry barriers and
    # tail semaphore-clear/barrier butterfly cost ~20 us on a ~7 us kernel.
    nc = bacc.Bacc("TRN2", target_bir_lowering=False, debug=False)
    x = nc.dram_tensor("x", [P, F], mybir.dt.float32, kind="ExternalInput")
    # output is the per-chunk grand totals on a single partition: the final
    # cross-partition sum runs on PE (ones-matmul), so the store to HBM is a
    # single 12-byte descriptor instead of 128 4-byte ones (~5 us cheaper
    # completion wait)
    out = nc.dram_tensor("out", [1, nch], mybir.dt.float32, kind="ExternalOutput")
    with ExitStack() as ctx:
        buf = ctx.enter_context(nc.sbuf_tensor([P, F], mybir.dt.float32))
        stats = ctx.enter_context(nc.sbuf_tensor([P, nch], mybir.dt.float32))
        ones = ctx.enter_context(nc.sbuf_tensor([P, 1], mybir.dt.float32))
        row = ctx.enter_context(nc.sbuf_tensor([1, nch], mybir.dt.float32))
        psum = ctx.enter_context(nc.psum_tensor([1, nch], mybir.dt.float32))
        # one completion semaphore per load: a DMA's 16 increments come from
        # 16 SDMA engines independently, so cumulative thresholds on a shared
        # semaphore would be racy across back-to-back DMAs
        dma_sems = [
            ctx.enter_context(nc.semaphore(f"dma_sem{i}")) for i in range(nch)
        ]
        out_sem = ctx.enter_context(nc.semaphore())
        vsem = ctx.enter_context(nc.semaphore())
        ones_sem = ctx.enter_context(nc.semaphore())
        psem = ctx.enter_context(nc.semaphore())
        block = ctx.enter_context(nc.Block())

        @block.sync
        def _(sync):
            for i, ((a, b), sem) in enumerate(zip(chunks, dma_sems)):
                if DUAL_RING and i % 2 == 1:
                    continue
                sync.dma_start(out=buf[:, a:b], in_=x[:, a:b]).then_inc(sem, 16)
            sync.wait_ge(vsem, nch + 1)
            sync.dma_start(out=out[:], in_=row[:]).then_inc(out_sem, 16)
            if WAIT_OUT:
                # drain: the NEFF must not finish before the HBM write lands
                sync.wait_ge(out_sem, 16)

        if DUAL_RING:

            @block.scalar
            def _(scalar):
                for i, ((a, b), sem) in enumerate(zip(chunks, dma_sems)):
                    if i % 2 == 0:
                        continue
                    scalar.dma_start(out=buf[:, a:b], in_=x[:, a:b]).then_inc(
                        sem, 16
                    )

        @block.vector
        def _(vector):
            nc.vector.memset(ones[:], 1.0).then_inc(ones_sem, 1)
            for i, ((a, b), sem) in enumerate(zip(chunks, dma_sems)):
                vector.wait_ge(sem, 16)
                nc.vector.reduce_sum(
                    stats[:, i : i + 1], buf[:, a:b], axis=mybir.AxisListType.X
                ).then_inc(vsem, 1)
            # copy PE's cross-partition totals PSUM -> SBUF for the store
            vector.wait_ge(psem, 1)
            nc.vector.tensor_copy(row[:], psum[:]).then_inc(vsem, 1)

        @block.tensor
        def _(tensor):
            tensor.wait_ge(ones_sem, 1)
            tensor.wait_ge(vsem, nch)
            nc.tensor.matmul(
                psum[:], ones[:], stats[:], start=True, stop=True
            ).then_inc(psem, 1)

    nc.compile()
    return nc


def _get_nc():
    if "nc" not in _CACHE:
        _CACHE["nc"] = _build_program()
    return _CACHE["nc"]


def _ensure_trace_support():
    """BASS_TRACE=1 routes run_bass_kernel_spmd through the NTFF profiling
    path, which imports antenv.axon_hooks (absent on some agent images) and
    uploads artifacts to a share (unreachable in sandboxes).  Fill those gaps
    so a profiling harness doesn't crash the kernel; no-op on images where
    the real hooks module exists."""
    import os
    import sys
    import types

    try:
        import antenv.axon_hooks  # noqa: F401
    except ImportError:
        try:
            import antenv
        except ImportError:
            return
        mod = types.ModuleType("antenv.axon_hooks")
        holder = [None]
        mod.set_axon_ntff_profile_hook = lambda h: holder.__setitem__(0, h)
        mod.get_axon_ntff_profile_hook = lambda: holder[0]
        sys.modules["antenv.axon_hooks"] = mod
        antenv.axon_hooks = mod
        try:
            from trn_agent_boot.trn_boot import _ntff_profile_via_ctypes

            so = "/opt/axon/libaxon_pjrt.so"
            if os.path.exists(so):
                mod.set_axon_ntff_profile_hook(_ntff_profile_via_ctypes(so))
        except Exception:
            pass

        import concourse.bass_utils as bu

        if not getattr(bu.upload_artifacts, "_safe_wrapped", False):
            orig = bu.upload_artifacts

            def safe_upload(tmpdir):
                try:
                    return orig(tmpdir)
                except Exception:
                    return tmpdir

            safe_upload._safe_wrapped = True
            bu.upload_artifacts = safe_upload


def _run_device_sums(area, trace=False, **kwargs):
    """Returns (sum over the first DEV_ELEMS of every shard, BassKernelResults)."""
    from concourse.bass_utils import run_bass_kernel_spmd

    _ensure_trace_support()

    nc = _get_nc()
    area = np.ascontiguousarray(area, dtype=np.float32)
    in_maps = [
        {"x": area[c * SHARD : c * SHARD + DEV_ELEMS].reshape(P, F)}
        for c in range(NCORES)
    ]
    res = run_bass_kernel_spmd(
        nc, in_maps, core_ids=list(range(NCORES)), trace=trace, **kwargs
    )
    dev_sum = float(
        sum(r["out"].astype(np.float64).sum() for r in res.results)
    )
    return dev_sum, res


def _minmod(a, b):
    if a * b > 0.0:
        return np.sign(a) * min(abs(a), abs(b))
    return 0.0


def _epilogue(total_sum, a3, s):
    """Scalar infiltration step + outlet-node MUSCL update (float64 host math).

    a3 = [A[N-3], A[N-2], A[N-1]]; s = dict of the scalar inputs.
    """
    mean = total_sum / N
    surface_head = mean / s["WID"]
    dtheta = max(s["theta_s"] - s["theta_current"], 0.0)
    f_cap = s["Ks"] * (
        1.0 + (s["psi"] + surface_head) * dtheta / max(s["F_cumulative"], EPS)
    )
    supply = s["rain_rate"] + surface_head / max(s["dt_s"], EPS)
    infil_rate = max(min(supply, f_cap), 0.0)
    infil_depth = infil_rate * s["dt_s"]

    net_rain = max(s["rain_rate"] - infil_rate, 0.0)
    q_lat = net_rain * s["WID"]

    # MUSCL faces at the last two cells.  At the outlet dA_p = 0 so the
    # minmod slope there is 0 and A_face[N-1] = max(A[N-1], 0).
    slope_m2 = _minmod(a3[1] - a3[0], a3[2] - a3[1])
    a_face_m2 = max(a3[1] + 0.5 * slope_m2, 0.0)
    a_face_m1 = max(a3[2], 0.0)
    coef = np.sqrt(s["SL"]) / (s["MAN"] * s["WID"] ** (2.0 / 3.0))
    q_face_m2 = a_face_m2 ** (5.0 / 3.0) * coef
    q_face_m1 = a_face_m1 ** (5.0 / 3.0) * coef

    a_next_last = max(
        a3[2] + s["dt_s"] * (q_lat - (q_face_m1 - q_face_m2) / s["dx"]), 0.0
    )
    outflow_q = a_next_last ** (5.0 / 3.0) * coef
    return np.array([outflow_q, infil_rate, infil_depth], dtype=np.float32)


def kernel(**inputs):
    area = np.asarray(inputs["area"], dtype=np.float32)
    assert area.shape == (N,), area.shape
    s = {
        k: float(np.asarray(v))
        for k, v in inputs.items()
        if k != "area"
    }

    dev_sum, _ = _run_device_sums(area)
    tail_sum = float(
        sum(
            area[c * SHARD + DEV_ELEMS : (c + 1) * SHARD].astype(np.float64).sum()
            for c in range(NCORES)
        )
    )
    total = dev_sum + tail_sum
    return _epilogue(total, area[-3:].astype(np.float64), s)
